# revision 21
# baseline (speedup 1.0000x reference)
"""Banded additive (Bahdanau) attention on 8 TRN2 NeuronCores.

Reference computation (B=2, L=1024, D=256, U=32, width 128, keys j in [i-64, i+63]):
    q = x @ Wt; k = x @ Wx
    e[b,i,j] = exp(Wa . tanh(q_i + k_j + bh) + ba) * band(i,j)
    v = (e / (sum_j e + eps)) @ x

Sharding: core = b*4 + chunk handles batch b, queries [chunk*256, chunk*256+256).
Each core receives a 384-row halo of x (queries +-64), so the band is fully
local: no collectives.

Algorithm per core (raw Bass, explicit semaphores — the walrus build here
only encodes ONE sync-wait per instruction, so Tile's attached waits don't
compile; standalone wait_ge instructions are unlimited):
  - all constants arrive in ONE blob DMA [128, TOT] bf16
  - q/k projections on PE with 4x unit-replicated weights (m=128)
  - staggered key replica kst[32r+u, c] = k[c+r, u] via 4 DVE copies
  - banded scores in offset space, transposed: ET[i, d] (d = 4b + r) via
    sliding DVE adds (one 3D-AP op per 4-block chunk), ACT tanh, and PE
    matmuls with the H-chunk as stationary operand and block-diag Wa4 [128,4]
    as the moving side -> ET PSUM cols [4b, 4b+4)
  - exp on ACT, band-mask multiply + row-sum + 1/(s+eps) on DVE
  - band -> dense[i, j_local] DRAM scratch via one diagonal-offset DMA
    (rows are contiguous W-wide segments), read back transposed (DMA
    transpose) as [j, i] tiles = stationary operands of the value matmul
  - value matmul on PE (contract over j), scale by 1/(s+eps) on DVE, store
"""

import sys

for _p in ("/opt/trn_rl_repo",):
    if _p not in sys.path:
        sys.path.insert(0, _p)

import ml_dtypes
import numpy as np

import concourse.bass as bass
from concourse import mybir
from concourse.bass_utils import run_bass_kernel_spmd

B, L, D, U = 2, 1024, 256, 32
W = 128            # attention width (band)
CHUNK = 256        # queries per core
NH = 384           # halo'd key rows per core: 64 + 256 + 64
G = 4              # unit-replication groups (4*32 = 128 partitions)
NBLK = W // G      # 32 offset blocks of 4
BPC = 4            # offset blocks fused per DVE/ACT op ("chunk")
NCH = NBLK // BPC  # 8 chunks
EPS = 1e-7

BF = mybir.dt.bfloat16
F32 = mybir.dt.float32
NPBF = ml_dtypes.bfloat16

# blob column layout (bf16 elements, per partition p)
OFF_XT = 0                  # 2 tiles x 384: xT[128t+p, :]
OFF_XH = OFF_XT + 2 * NH    # 3 tiles x 256: xh[128t+p, :]
OFF_WT = OFF_XH + 3 * D     # 2 tiles x 128: wt4[128t+p, :]
OFF_WX = OFF_WT + 2 * 128   # 2 tiles x 128
OFF_WA = OFF_WX + 2 * 128   # 4: wa4[p, :]
OFF_MASK = OFF_WA + G       # 2 tiles x 128: mask[128t+p, :]
OFF_BH = OFF_MASK + 2 * W   # 2 (f32 as bf16 pair): bh4[p]
TOT = OFF_BH + 2


def build_bass():
    nc = bass.Bass()
    blob = nc.declare_dram_parameter("blob", [128, TOT], BF, isOutput=False)
    out = nc.declare_dram_parameter("out", [CHUNK, D], F32, isOutput=True)
    dense = nc.dram_tensor("dense", [CHUNK, NH], BF)

    ctxs = []

    def sb(name, shape, dtype=BF):
        cm = nc.sbuf_tensor(name, shape, dtype)
        t = cm.__enter__()
        ctxs.append(cm)
        return t

    def ps(name, shape):
        cm = nc.psum_tensor(name, shape, F32)
        t = cm.__enter__()
        ctxs.append(cm)
        return t

    def sem(name):
        cm = nc.semaphore(name)
        s = cm.__enter__()
        ctxs.append(cm)
        return s

    cb = sb("cb", [128, TOT])
    zero_sb = sb("zero_sb", [128, 2 * NH])
    bh4_sb = sb("bh4_sb", [128, 1], F32)
    q_sb = sb("q_sb", [128, CHUNK])
    k_sb = sb("k_sb", [128, NH])
    kst = sb("kst", [128, NH])
    s_sb = [sb(f"s{c}", [128, BPC * CHUNK]) for c in range(NCH)]
    h_sb = [sb(f"h{c}", [128, BPC * CHUNK]) for c in range(NCH)]
    pT_raw = sb("pT_raw", [128, 2, W])
    pT = sb("pT", [128, 2, W])
    ssum = sb("ssum", [128, 2], F32)
    rcp = sb("rcp", [128, 2], F32)
    dj = [sb(f"dj{t}", [128, CHUNK]) for t in range(3)]
    o_sb = [sb(f"o{it}", [128, D], F32) for it in range(2)]

    q_ps = ps("q_ps", [128, CHUNK])
    k_ps = ps("k_ps", [128, NH])
    et_ps = ps("et_ps", [128, 2, W])
    v_ps = [ps(f"v{it}_ps", [128, D]) for it in range(2)]

    S_blob = sem("S_blob")
    S_zero = sem("S_zero")
    S_qk = sem("S_qk")
    S_bh = sem("S_bh")
    S_s = sem("S_s")
    S_h = sem("S_h")
    S_et = sem("S_et")
    S_praw = sem("S_praw")
    S_pt = sem("S_pt")
    S_diag = sem("S_diag")
    S_dj = sem("S_dj")
    S_v = sem("S_v")
    S_o = sem("S_o")
    S_out = sem("S_out")

    def cbs(off, n):
        return cb[:, off:off + n]

    xT_t = [cbs(OFF_XT + NH * t, NH) for t in range(2)]
    xh_t = [cbs(OFF_XH + D * t, D) for t in range(3)]
    wt_t = [cbs(OFF_WT + 128 * t, 128) for t in range(2)]
    wx_t = [cbs(OFF_WX + 128 * t, 128) for t in range(2)]
    wa4 = cbs(OFF_WA, G)
    mask_t = [cbs(OFF_MASK + W * t, W) for t in range(2)]
    bh4_raw = cbs(OFF_BH, 2).bitcast(F32)

    with nc.Block() as block:

        @block.sync
        def _(sync):
            sync.dma_start(out=cb[:], in_=blob[:]).then_inc(S_blob, 16)
            sync.wait_ge(S_diag, 16)
            for t in range(3):
                sync.dma_start_transpose(
                    out=dj[t][:], in_=dense[:, 128 * t:128 * (t + 1)]
                ).then_inc(S_dj, 16)

        @block.gpsimd
        def _(gpsimd):
            gpsimd.memset(zero_sb[:], 0.0)
            dense_zap = bass.AP(tensor=dense, offset=0,
                                ap=[[2 * NH, 128], [1, 2 * NH]])
            gpsimd.dma_start(out=dense_zap, in_=zero_sb[:]).then_inc(S_zero, 16)
            gpsimd.wait_ge(S_pt, 2)
            gpsimd.wait_ge(S_zero, 16)
            # scatter band -> dense[i, j_local]: row (128*it + p), cols [i, i+W)
            # flat offset = (NH+1)*(128*it + p) + d  (contiguous W-runs)
            diag = bass.AP(tensor=dense, offset=0,
                           ap=[[NH + 1, 128], [128 * (NH + 1), 2], [1, W]])
            gpsimd.dma_start(out=diag, in_=pT[:]).then_inc(S_diag, 16)
            gpsimd.wait_ge(S_o, 2)
            for it in range(2):
                gpsimd.dma_start(out=out[128 * it:128 * (it + 1), :],
                                 in_=o_sb[it][:]).then_inc(S_out, 16)
            gpsimd.wait_ge(S_out, 32)

        @block.tensor
        def _(pe):
            pe.wait_ge(S_blob, 16)
            pe.matmul(q_ps[:], lhsT=wt_t[0], rhs=xT_t[0][:, 64:64 + CHUNK],
                      start=True, stop=False)
            pe.matmul(q_ps[:], lhsT=wt_t[1], rhs=xT_t[1][:, 64:64 + CHUNK],
                      start=False, stop=True).then_inc(S_qk, 1)
            pe.matmul(k_ps[:], lhsT=wx_t[0], rhs=xT_t[0], start=True, stop=False)
            pe.matmul(k_ps[:], lhsT=wx_t[1], rhs=xT_t[1],
                      start=False, stop=True).then_inc(S_qk, 1)
            # ET[i, 4b+r] = sum_u h[(r,u), i] * Wa4[(r,u), r]
            for c in range(NCH):
                pe.wait_ge(S_h, c + 1)
                for t in range(BPC):
                    b = BPC * c + t
                    for it in range(2):
                        mm = pe.matmul(
                            et_ps[:, it, G * b:G * (b + 1)],
                            lhsT=h_sb[c][:, CHUNK * t + 128 * it:CHUNK * t + 128 * (it + 1)],
                            rhs=wa4, start=True, stop=True)
                mm.then_inc(S_et, 1)
            pe.wait_ge(S_dj, 48)
            for it in range(2):
                for t in range(3):
                    mm = pe.matmul(v_ps[it][:], lhsT=dj[t][:, 128 * it:128 * (it + 1)],
                                   rhs=xh_t[t], start=(t == 0), stop=(t == 2))
                mm.then_inc(S_v, 1)

        @block.scalar
        def _(act):
            act.wait_ge(S_bh, 1)
            for c in range(NCH):
                act.wait_ge(S_s, c + 1)
                act.activation(out=h_sb[c][:], in_=s_sb[c][:],
                               func=mybir.ActivationFunctionType.Tanh,
                               bias=bh4_sb[:]).then_inc(S_h, 1)
            act.wait_ge(S_et, NCH)
            for it in range(2):
                act.activation(out=pT_raw[:, it, :], in_=et_ps[:, it, :],
                               func=mybir.ActivationFunctionType.Exp
                               ).then_inc(S_praw, 1)

        @block.vector
        def _(dve):
            dve.wait_ge(S_blob, 16)
            dve.tensor_copy(out=bh4_sb[:], in_=bh4_raw).then_inc(S_bh, 1)
            dve.wait_ge(S_qk, 2)
            dve.tensor_copy(out=q_sb[:], in_=q_ps[:])
            dve.tensor_copy(out=k_sb[:], in_=k_ps[:])
            for r in range(G):
                dve.tensor_copy(out=kst[32 * r:32 * (r + 1), 0:381],
                                in_=k_sb[32 * r:32 * (r + 1), r:r + 381])
            q_ap = q_sb[:]
            kst_ap = kst[:]
            for c in range(NCH):
                in0 = bass.AP(tensor=kst_ap.tensor, offset=kst_ap.offset + G * BPC * c,
                              ap=[kst_ap.ap[0], [G, BPC], [1, CHUNK]])
                in1 = bass.AP(tensor=q_ap.tensor, offset=q_ap.offset,
                              ap=[q_ap.ap[0], [0, BPC], [1, CHUNK]])
                s3 = bass.AP(tensor=s_sb[c], offset=0,
                             ap=[[BPC * CHUNK, 128], [CHUNK, BPC], [1, CHUNK]])
                dve.tensor_add(out=s3, in0=in0, in1=in1).then_inc(S_s, 1)
            for it in range(2):
                dve.wait_ge(S_praw, it + 1)
                dve.tensor_mul(out=pT[:, it, :], in0=pT_raw[:, it, :],
                               in1=mask_t[it]).then_inc(S_pt, 1)
                dve.reduce_sum(out=ssum[:, it:it + 1], in_=pT[:, it, :],
                               axis=mybir.AxisListType.X)
                dve.tensor_scalar_add(out=ssum[:, it:it + 1], in0=ssum[:, it:it + 1],
                                      scalar1=float(EPS))
                dve.reciprocal(out=rcp[:, it:it + 1], in_=ssum[:, it:it + 1])
            dve.wait_ge(S_v, 2)
            for it in range(2):
                dve.tensor_scalar_mul(out=o_sb[it][:], in0=v_ps[it][:],
                                      scalar1=rcp[:, it:it + 1]).then_inc(S_o, 1)

    for cm in reversed(ctxs):
        cm.__exit__(None, None, None)
    return nc


def make_in_maps(x, Wt, Wx, bh, Wa, ba):
    x = np.asarray(x, np.float32)
    Wt = np.asarray(Wt, np.float32)
    Wx = np.asarray(Wx, np.float32)
    bh = np.asarray(bh, np.float32).reshape(U)
    Wa = np.asarray(Wa, np.float32).reshape(U)
    ba = np.asarray(ba, np.float32).reshape(1)

    wt4 = np.tile(Wt, (1, G)).astype(NPBF)          # [D, 128]
    wx4 = np.tile(Wx, (1, G)).astype(NPBF)
    wa4 = np.zeros((128, G), np.float32)
    for r in range(G):
        wa4[32 * r:32 * (r + 1), r] = Wa
    wa4 = wa4.astype(NPBF)
    bh4 = np.ascontiguousarray(np.tile(bh, G).reshape(128, 1), np.float32)
    bh4_bits = bh4.view(np.uint16).view(NPBF)       # [128, 2] raw f32 bytes

    dd = np.arange(W)[None, :]
    ii = np.arange(CHUNK)[:, None]

    in_maps = []
    for core in range(8):
        b, ch = divmod(core, 4)
        lo = ch * CHUNK - 64
        xpad = np.zeros((NH, D), np.float32)
        s0, s1 = max(0, lo), min(L, lo + NH)
        xpad[s0 - lo:s1 - lo] = x[b, s0:s1]
        j = lo + ii + dd
        m = (((j >= 0) & (j < L)).astype(np.float32) * np.exp(ba[0])).astype(NPBF)
        xT = np.ascontiguousarray(xpad.T).astype(NPBF)   # [D, NH]
        xh = xpad.astype(NPBF)                           # [NH, D]

        blob = np.zeros((128, TOT), NPBF)
        for t in range(2):
            blob[:, OFF_XT + NH * t:OFF_XT + NH * (t + 1)] = xT[128 * t:128 * (t + 1)]
        for t in range(3):
            blob[:, OFF_XH + D * t:OFF_XH + D * (t + 1)] = xh[128 * t:128 * (t + 1)]
        for t in range(2):
            blob[:, OFF_WT + 128 * t:OFF_WT + 128 * (t + 1)] = wt4[128 * t:128 * (t + 1)]
            blob[:, OFF_WX + 128 * t:OFF_WX + 128 * (t + 1)] = wx4[128 * t:128 * (t + 1)]
        blob[:, OFF_WA:OFF_WA + G] = wa4
        for t in range(2):
            blob[:, OFF_MASK + W * t:OFF_MASK + W * (t + 1)] = m[128 * t:128 * (t + 1)]
        blob[:, OFF_BH:OFF_BH + 2] = bh4_bits
        in_maps.append({"blob": blob})
    return in_maps


def assemble(results):
    out = np.zeros((B, L, D), np.float32)
    for core in range(8):
        b, ch = divmod(core, 4)
        out[b, ch * CHUNK:(ch + 1) * CHUNK, :] = results[core]["out"]
    return out


def kernel(x, Wt, Wx, bh, Wa, ba):
    nc = build_bass()
    in_maps = make_in_maps(x, Wt, Wx, bh, Wa, ba)
    res = run_bass_kernel_spmd(nc, in_maps, core_ids=list(range(8)))
    return assemble(res.results)


if __name__ == "__main__":
    rng = np.random.default_rng(0)
    glorot = lambda shape: rng.standard_normal(shape, np.float32) * np.sqrt(2.0 / (shape[0] + shape[-1]))
    inputs = {
        "x": rng.standard_normal((B, L, D), np.float32),
        "Wt": glorot((D, U)), "Wx": glorot((D, U)),
        "bh": np.zeros(U, np.float32), "Wa": glorot((U, 1)),
        "ba": np.zeros(1, np.float32),
    }
    out = kernel(**inputs)
    print("kernel ran, out shape", out.shape, "finite:", np.isfinite(out).all())
